# revision 22
# baseline (speedup 1.0000x reference)
"""CLRNet SimOTA assignment kernel for Trainium2 (Bass/Tile), pair-split.

Contract: kernel(**inputs) takes FULL inputs (preds [4,4096,78], targets
[4,32,78], masks [4,32], img_w, img_h) and returns (assigned [4,4096] bool,
matched [4,4096] int32) exactly like the reference.

Sharding: each image b runs on the core pair (2b, 2b+1); each core handles
2048 priors (16 tiles of 128). The per-image global statistics (masked maxes
of the three distance matrices, per-target top-4 of -distance for SimOTA
dyn_k, per-target top-4 of -cost for the selection threshold ck) are merged
across the pair with two tiny AllGather collectives (pairs replica groups).
A dummy AllGather fired at kernel start prepays the ~30us ncfw collective
warmup while the distance scan runs, so the two real exchanges each cost
only a few us.

Key design (driven by measured TRN2 instruction costs):
  - D[n,t] = sum_s valid*|px[n,s]-tx[t,s]| via ONE custom-DVE scan op per
    prior tile (prefix over the (t,s) free dim, step-0 output AP keeps only
    block ends). Invalid points contribute px, removed by P = px @ invmask^T
    on the idle PE.
  - top-4 iou priors = 4 smallest-distance priors (iou monotone in D), so
    all top-4 selections are Max8 on negated values + threshold, no indices.
  - start/theta distance matrices computed on the otherwise-idle GpSimd +
    Scalar engines while the DVE runs the scan.
  - No +/-inf: +/-1e30 sentinels.
"""

import os
import sys

import numpy as np

for _p in ("/opt/trn_rl_repo", "/root/.axon_site/_ro/trn_rl_repo"):
    if os.path.isdir(_p) and _p not in sys.path:
        sys.path.insert(0, _p)

import concourse.bacc as bacc  # noqa: E402
import concourse.bass as bass  # noqa: E402
import concourse.mybir as mybir  # noqa: E402
from concourse import bass_isa, dve_ops  # noqa: E402
from concourse.dve_spec import AluOp as DAlu  # noqa: E402
from concourse.dve_spec import Bin, Spec, Src0, Src1, lower, scan  # noqa: E402
from concourse.dve_uop import DveOpSpec  # noqa: E402
from concourse.tile import TileContext  # noqa: E402

F32 = mybir.dt.float32
I32 = mybir.dt.int32
U8 = mybir.dt.uint8
Alu = mybir.AluOpType
Act = mybir.ActivationFunctionType
AX = mybir.AxisListType

N = 4096
NH = 2048  # per-core prior count
TFULL = 32
S = 72
NT = NH // 128  # 16 prior tiles of 128 per core
BIG = 1.0e30
EPS = 1.0e-12
GROUPS = [[0, 1], [2, 3], [4, 5], [6, 7]]


def _register_absdiff_scan():
    """Custom DVE op: prefix-sum of |in0 - in1| along the free dim."""
    name = "ABSDIFF_SCAN_CLR"
    for op in dve_ops.OPS:
        if op.name == name:
            return op

    def _ref(in0, in1, s0, s1, imm2):
        a = in0.astype(np.float32).reshape(in0.shape[0], -1)
        b = in1.astype(np.float32).reshape(in1.shape[0], -1)
        d = np.abs(a - b)
        return np.cumsum(d, axis=-1).astype(np.float32)

    spec = Spec(
        body=scan(DAlu.ADD, Bin(DAlu.ABSOLUTE_DIFF, Src0, Src1)),
        reference=_ref,
    )
    shas = {}
    for ver in ("v3", "v4"):
        u = lower(spec, ver=ver)
        shas[ver] = DveOpSpec(name=name, opcode=0, uops=u, rd1_en=True).sha(ver)
    op = dve_ops.DveOp(name, spec, subdim=False, uops_sha=shas)
    dve_ops.OPS.append(op)
    dve_ops.CUSTOM_DVE_SPECS[name] = spec
    dve_ops._SUB_OPCODE_FOR_NAME[name] = (
        dve_ops._CUSTOM_DVE_ROW_BASE + len(dve_ops.OPS) - 1
    )
    return op


ABSDIFF_SCAN = _register_absdiff_scan()


def build(img_w: float, Tc: int) -> bass.Bass:
    T = Tc
    NTT = NT * T
    TS = T * S
    nc = bacc.Bacc("TRN2", target_bir_lowering=False, debug=False, num_devices=8)

    pxall_d = nc.dram_tensor("pxall", [128, NT * S], F32, kind="ExternalInput")
    pcols_d = nc.dram_tensor("pcols", [128, 5 * NT], F32, kind="ExternalInput")
    pcolsp_d = nc.dram_tensor("pcolsp", [128, 5 * NT], F32, kind="ExternalInput")
    id_d = nc.dram_tensor("ident", [128, 128], F32, kind="ExternalInput")
    txb_d = nc.dram_tensor("txball", [128, TS], F32, kind="ExternalInput")
    trowsb_d = nc.dram_tensor("trowsb", [128, 8 * T], F32, kind="ExternalInput")
    invms_d = nc.dram_tensor("invms", [S, T], F32, kind="ExternalInput")
    tcols_d = nc.dram_tensor("tcols", [T, 16], F32, kind="ExternalInput")
    ot_d = nc.dram_tensor("origt", [128, T], F32, kind="ExternalInput")
    asn_d = nc.dram_tensor("assigned", [NH], U8, kind="ExternalOutput")
    mat_d = nc.dram_tensor("matched", [NH], I32, kind="ExternalOutput")

    with TileContext(nc) as tc:
        with (
            tc.tile_pool(name="const", bufs=1) as constp,
            tc.tile_pool(name="setup", bufs=1) as setp,
            tc.tile_pool(name="big", bufs=1) as bigp,
            tc.tile_pool(name="stile", bufs=2) as sp,
            tc.tile_pool(name="small", bufs=2) as smp,
            tc.tile_pool(name="psT", bufs=2, space="PSUM") as psT,
            tc.tile_pool(name="psP", bufs=2, space="PSUM") as psP,
            tc.tile_pool(name="psF", bufs=2, space="PSUM") as psF,
            tc.tile_pool(name="psB", bufs=2, space="PSUM") as psB,
            tc.tile_pool(name="dram", bufs=1, space="DRAM") as dram,
        ):
            v = nc.vector
            sc = nc.scalar
            gp = nc.gpsimd
            te = nc.tensor

            # ---------------- inputs (all contiguous, host-prepped) ------
            txb = bigp.tile([128, TS], F32)
            half = TS // 2
            sc.dma_start(
                out=txb[:, 0:half], in_=bass.AP(txb_d, 0, [[TS, 128], [1, half]])
            )
            nc.sync.dma_start(
                out=txb[:, half:TS],
                in_=bass.AP(txb_d, half, [[TS, 128], [1, TS - half]]),
            )
            trowsb = setp.tile([128, 8 * T], F32)
            nc.sync.dma_start(out=trowsb[:], in_=trowsb_d.ap())
            ones = constp.tile([1, 128], F32)
            v.memset(ones[:], 1.0)

            # PE (ones-column matmul) + scalar-copy broadcast for ck.
            def pe_bcast(dst_ap, src_row_ap, width, name):
                k = 0
                while k < width:
                    w = min(512, width - k)
                    pb = psB.tile([128, 512], F32, tag="pb", name=f"pb_{name}{k}")
                    te.matmul(
                        pb[:, 0:w],
                        ones[:],
                        src_row_ap[:, k : k + w],
                        start=True,
                        stop=True,
                    )
                    sc.copy(dst_ap[:, k : k + w], pb[:, 0:w])
                    k += w

            pxall = bigp.tile([128, NT * S], F32)
            CH = 4
            chw = (NT // CH) * S
            for c in range(CH):
                gp.dma_start(
                    out=pxall[:, c * chw : (c + 1) * chw],
                    in_=bass.AP(pxall_d, c * chw, [[NT * S, 128], [1, chw]]),
                )
            ident = constp.tile([128, 128], F32)
            nc.sync.dma_start(out=ident[:], in_=id_d.ap())
            pcols = setp.tile([128, 5 * NT], F32)
            nc.sync.dma_start(out=pcols[:], in_=pcols_d.ap())
            pcolsp = setp.tile([128, 5 * NT], F32)
            nc.sync.dma_start(out=pcolsp[:], in_=pcolsp_d.ap())
            invmC = setp.tile([S, T], F32)
            nc.sync.dma_start(out=invmC[:], in_=invms_d.ap())
            tcols = setp.tile([T, 16], F32)
            nc.sync.dma_start(out=tcols[:], in_=tcols_d.ap())
            origt = constp.tile([128, T], F32)
            nc.sync.dma_start(out=origt[:], in_=ot_d.ap())

            # views of the broadcast target rows
            labb = trowsb[:, 0 * T : 1 * T]
            sxtb = trowsb[:, 1 * T : 2 * T]
            sytb = trowsb[:, 2 * T : 3 * T]
            thtb = trowsb[:, 3 * T : 4 * T]
            invvlenb = trowsb[:, 5 * T : 6 * T]
            maskminb = trowsb[:, 6 * T : 7 * T]
            bigmask = trowsb[:, 7 * T : 8 * T]
            # per-target columns
            iotaq = tcols[:, 0:4]
            thirty = tcols[:, 4:5]
            thirty_eps = tcols[:, 5:6]
            nvlenp = tcols[:, 6:7]
            maskcol = tcols[:, 7:8]
            mb = tcols[:, 8:9]
            # pred feature columns
            sxp = pcols[:, 2 * NT : 3 * NT]
            syp = pcols[:, 3 * NT : 4 * NT]
            thp = pcols[:, 4 * NT : 5 * NT]

            def b3(ap2d, axis):
                if axis == "tile":
                    return ap2d.unsqueeze(1).broadcast_to([128, NT, T])
                return ap2d.unsqueeze(2).broadcast_to([128, NT, T])

            # ------- start/theta distance matrices on GpSimd + Scalar ----
            # (runs while the DVE is busy with the scan below)
            # own half in cols 0:NTT, peer half in NTT:2*NTT
            sd_all = bigp.tile([128, 2 * NTT], F32)
            dy_all = bigp.tile([128, 2 * NTT], F32)
            td_all = bigp.tile([128, 2 * NTT], F32)
            def sdtd_subs(h, pcv):
                hs = h * NTT
                sxh = pcv[:, 2 * NT : 3 * NT]
                syh = pcv[:, 3 * NT : 4 * NT]
                thh = pcv[:, 4 * NT : 5 * NT]
                s3 = sd_all[:, hs : hs + NTT].rearrange("p (i t) -> p i t", t=T)
                y3 = dy_all[:, hs : hs + NTT].rearrange("p (i t) -> p i t", t=T)
                t3 = td_all[:, hs : hs + NTT].rearrange("p (i t) -> p i t", t=T)
                gp.tensor_tensor(
                    out=s3, in0=b3(sxtb, "tile"), in1=b3(sxh, "t"), op=Alu.subtract
                )
                gp.tensor_tensor(
                    out=y3, in0=b3(sytb, "tile"), in1=b3(syh, "t"), op=Alu.subtract
                )
                gp.tensor_tensor(
                    out=t3, in0=b3(thtb, "tile"), in1=b3(thh, "t"), op=Alu.subtract
                )

            def sdtd_tail(h):
                hs = h * NTT
                sc.activation(
                    sd_all[:, hs : hs + NTT], sd_all[:, hs : hs + NTT], Act.Square
                )
                sc.activation(
                    dy_all[:, hs : hs + NTT], dy_all[:, hs : hs + NTT], Act.Square
                )
                gp.tensor_tensor(
                    out=sd_all[:, hs : hs + NTT],
                    in0=sd_all[:, hs : hs + NTT],
                    in1=dy_all[:, hs : hs + NTT],
                    op=Alu.add,
                )
                sc.activation(
                    sd_all[:, hs : hs + NTT], sd_all[:, hs : hs + NTT], Act.Sqrt
                )
                sc.activation(
                    td_all[:, hs : hs + NTT], td_all[:, hs : hs + NTT], Act.Abs
                )

            # own+peer subtractions early on GpSimd (pre-warm); only the OWN
            # square/sqrt tail runs pre-loop on the scalar queue — the peer
            # tail is deferred past the scan loop so it cannot delay the
            # smat/P copies that feed the in-loop D assembly.
            sdtd_subs(0, pcols)
            sdtd_subs(1, pcolsp)
            sdtd_tail(0)

            # ---------------- focal-loss E columns (scalar engine) -------
            ceps = setp.tile([128, 1], F32)
            v.memset(ceps[:], EPS)
            c1peps = setp.tile([128, 1], F32)
            v.memset(c1peps[:], 1.0 + EPS)
            pr = setp.tile([128, 4 * NT], F32)
            l1 = setp.tile([128, 4 * NT], F32)
            l2 = setp.tile([128, 4 * NT], F32)
            q2 = setp.tile([128, 4 * NT], F32)
            p2 = setp.tile([128, 4 * NT], F32)
            for h, pcv in ((0, pcols), (1, pcolsp)):
                hs, he = h * 2 * NT, (h + 1) * 2 * NT
                sc.activation(pr[:, hs:he], pcv[:, 0 : 2 * NT], Act.Sigmoid)
                sc.activation(l1[:, hs:he], pr[:, hs:he], Act.Ln, bias=ceps[:], scale=1.0)
                sc.activation(
                    l2[:, hs:he], pr[:, hs:he], Act.Ln, bias=c1peps[:], scale=-1.0
                )
                sc.activation(
                    q2[:, hs:he], pr[:, hs:he], Act.Square, bias=1.0, scale=-1.0
                )
                sc.activation(p2[:, hs:he], pr[:, hs:he], Act.Square)

            # ---------------- warmup collective (prepays ncfw setup) -----
            # NOTE: a collective BLOCKS the gpsimd queue until it completes,
            # so it is triggered only after all early gpsimd compute.
            warm = constp.tile([1, 16], F32)
            v.memset(warm[:], 0.0)
            warm_in = dram.tile([1, 16], F32, name="warm_in")
            warm_out = dram.tile([1, 32], F32, name="warm_out")
            nc.sync.dma_start(warm_in[:], warm[:])
            gp.collective_compute(
                "AllGather",
                Alu.bypass,
                replica_groups=GROUPS,
                ins=[warm_in[:].opt()],
                outs=[warm_out[:].opt()],
            )

            # ---------------- phase 1: scans + P matmuls ----------------
            # D assembly + negdist fold run per 4-tile group on GpSimd/PE
            # WHILE the DVE scans later tiles: per-group ends tiles keep the
            # dependency tracking precise.
            GT = 4  # tiles per psum / assembly group
            NG = NT // GT
            dist_all = bigp.tile([128, 2 * NTT], F32)
            fold_nd = bigp.tile([128, 512], F32, tag="foldednd")
            P_all = bigp.tile([128, NTT], F32)
            ends_g = [None] * NG

            def b3g(ap2d):
                return ap2d.unsqueeze(1).broadcast_to([128, GT, T])

            for i in range(NT):
                g = i // GT
                if i % GT == 0:
                    ends_g[g] = bigp.tile(
                        [128, GT * (T + 1)], F32, name=f"ends{g}"
                    )
                    v.memset(ends_g[g][:], 0.0)
                eg = ends_g[g]
                px = pxall[:, i * S : (i + 1) * S]
                pxv = bass.AP(px.tensor, px.offset, [list(px.ap[0]), [0, T], [1, S]])
                # scan writes only each 72-block's final prefix (step-0 inner)
                endv = bass.AP(
                    eg.tensor,
                    eg.offset + (i % GT) * (T + 1) + 1,
                    [list(eg.ap[0]), [1, T], [0, S]],
                )
                v._custom_dve(ABSDIFF_SCAN, out=endv, in0=pxv, in1=txb[:])
                p_pxT = psT.tile([S, 128], F32, tag="tr")
                te.transpose(p_pxT[:], px, ident[:])
                smat = sp.tile([S, 128], F32, tag="smat")
                sc.copy(smat[:], p_pxT[:])
                if i % GT == 0:
                    pP = psP.tile([128, GT * T], F32, tag="pP", name=f"pP{g}")
                te.matmul(
                    pP[:, (i % GT) * T : (i % GT + 1) * T],
                    smat[:],
                    invmC[:],
                    start=True,
                    stop=True,
                )
                if i % GT == GT - 1:
                    gs, ge = g * GT * T, (g + 1) * GT * T
                    sc.copy(P_all[:, gs:ge], pP[:])
                    # D assembly for this group on GpSimd (overlaps the scan)
                    ehg = bass.AP(
                        eg.tensor, eg.offset + 1, [list(eg.ap[0]), [T + 1, GT], [1, T]]
                    )
                    elg = bass.AP(
                        eg.tensor, eg.offset, [list(eg.ap[0]), [T + 1, GT], [1, T]]
                    )
                    dgf = dist_all[:, gs:ge]
                    dg3 = dgf.rearrange("p (i t) -> p i t", t=T)
                    v.tensor_tensor(out=dg3, in0=ehg, in1=elg, op=Alu.subtract)
                    v.tensor_tensor(
                        out=dgf, in0=dgf, in1=P_all[:, gs:ge], op=Alu.subtract
                    )
                    v.tensor_tensor(out=dg3, in0=dg3, in1=b3g(invvlenb), op=Alu.mult)
                    # negdist fold for this group (PE + scalar copy)
                    psg = psF.tile([T, 512], F32, tag="psg", name=f"psgd{g}")
                    for j in range(GT):
                        te.transpose(
                            psg[:, j * 128 : (j + 1) * 128],
                            dist_all[:, (4 * g + j) * T : (4 * g + j + 1) * T],
                            ident[:],
                        )
                    sc.activation(
                        fold_nd[g * 32 : g * 32 + T, 0:512],
                        psg[:],
                        Act.Copy,
                        scale=-1.0,
                    )

            m1e = setp.tile([128, 4 * NT], F32)
            gp.tensor_tensor(out=m1e[:], in0=l1[:], in1=q2[:], op=Alu.mult)
            m2e = setp.tile([128, 4 * NT], F32)
            gp.tensor_tensor(out=m2e[:], in0=l2[:], in1=p2[:], op=Alu.mult)
            sc.activation(m2e[:], m2e[:], Act.Copy, scale=0.75)
            sc.activation(m1e[:], m1e[:], Act.Copy, scale=-0.25)
            ecols = setp.tile([128, 4 * NT], F32)
            gp.tensor_tensor(out=ecols[:], in0=m1e[:], in1=m2e[:], op=Alu.add)
            e0c = ecols[:, 0:NT]
            e0c2 = ecols[:, 2 * NT : 3 * NT]
            de_all = setp.tile([128, 2 * NT], F32)
            gp.tensor_tensor(
                out=de_all[:, 0:NT], in0=ecols[:, NT : 2 * NT], in1=e0c,
                op=Alu.subtract,
            )
            gp.tensor_tensor(
                out=de_all[:, NT : 2 * NT], in0=ecols[:, 3 * NT : 4 * NT], in1=e0c2,
                op=Alu.subtract,
            )

            sdtd_tail(1)

            # ---------------- local stats ------------------
            # per-partition masked maxes of the three matrices
            mk3 = b3(maskminb, "tile")

            def pp_max(acc, dst_ap, name):
                mm = smp.tile([128, NTT], F32, tag="statscratch")
                v.tensor_tensor(
                    out=mm[:].rearrange("p (i t) -> p i t", t=T),
                    in0=acc.rearrange("p (i t) -> p i t", t=T),
                    in1=mk3,
                    op=Alu.min,
                )
                v.tensor_reduce(dst_ap, mm[:], axis=AX.X, op=Alu.max)

            pmax = smp.tile([128, 4], F32, tag="pmax")
            pp_max(dist_all[:, 0:NTT], pmax[:, 0:1], "d")
            pp_max(sd_all[:, 0:NTT], pmax[:, 1:2], "s")
            pp_max(td_all[:, 0:NTT], pmax[:, 2:3], "t")

            def fold_tail(folded):
                cand = smp.tile([128, 8], F32, tag="cand8")
                v.max(out=cand[:], in_=folded[:])
                cg = smp.tile([T, 8 * (NT // 4)], F32, tag="cg")
                for c in range(NT // 4):
                    v.tensor_copy(cg[:, 8 * c : 8 * (c + 1)], cand[c * 32 : c * 32 + T, :])
                top8 = smp.tile([T, 8], F32, tag="top8")
                v.max(out=top8[:], in_=cg[:])
                return top8

            nd8 = fold_tail(fold_nd)

            # ---------------- exchange 1: maxes + dist top-4 -------------
            # payload [128, 8]: col0..2 per-partition d/s/t maxes,
            # cols 4:8 local top-4 of -distance (rows < T valid).
            P1 = smp.tile([128, 8], F32, tag="P1")
            v.memset(P1[:], -BIG)
            v.tensor_copy(P1[:, 0:3], pmax[:, 0:3])
            v.tensor_copy(P1[0:T, 4:8], nd8[:, 0:4])
            PW = NTT + 8
            cc1_in = dram.tile([128, PW], F32, name="cc1_in")
            cc1_out = dram.tile([256, PW], F32, name="cc1_out")
            sc.dma_start(cc1_in[:, 0:NTT], dist_all[:, 0:NTT])
            nc.sync.dma_start(cc1_in[:, NTT:PW], P1[:])
            gp.collective_compute(
                "AllGather",
                Alu.bypass,
                replica_groups=GROUPS,
                ins=[cc1_in[:].opt()],
                outs=[cc1_out[:].opt()],
            )
            R1 = smp.tile([128, 16], F32, tag="R1")
            o1 = cc1_out[:]
            nc.sync.dma_start(
                out=R1[:].rearrange("p (j f) -> p j f", j=2),
                in_=bass.AP(
                    o1.tensor, o1.offset + NTT, [[PW, 128], [128 * PW, 2], [1, 8]]
                ),
            )
            # gather blocks are rank-ordered (even core first), so recover the
            # peer's D symmetrically: peer = block0 + block1 - own.
            dsum = bigp.tile([128, NTT], F32, tag="dsum")
            sc.dma_start(
                out=dsum[:],
                in_=bass.AP(o1.tensor, o1.offset, [[PW, 128], [1, NTT]]),
            )
            nc.sync.dma_start(
                out=dist_all[:, NTT : 2 * NTT],
                in_=bass.AP(o1.tensor, o1.offset + 128 * PW, [[PW, 128], [1, NTT]]),
            )
            v.tensor_tensor(
                out=dist_all[:, NTT : 2 * NTT],
                in0=dist_all[:, NTT : 2 * NTT],
                in1=dsum[:],
                op=Alu.add,
            )
            v.tensor_tensor(
                out=dist_all[:, NTT : 2 * NTT],
                in0=dist_all[:, NTT : 2 * NTT],
                in1=dist_all[:, 0:NTT],
                op=Alu.subtract,
            )

            # ---- work not needing the exchange: cls matrix assembly ----
            cls_all = bigp.tile([128, 2 * NTT], F32)
            for h in (0, 1):
                hs = h * NTT
                c3v = cls_all[:, hs : hs + NTT].rearrange("p (i t) -> p i t", t=T)
                deh = de_all[:, h * NT : (h + 1) * NT]
                e0h = e0c if h == 0 else e0c2
                v.tensor_tensor(
                    out=c3v, in0=b3(labb, "tile"), in1=b3(deh, "t"), op=Alu.mult
                )
                v.tensor_tensor(out=c3v, in0=c3v, in1=b3(e0h, "t"), op=Alu.add)
                v.tensor_tensor(out=c3v, in0=c3v, in1=b3(bigmask, "tile"), op=Alu.add)

            # ---------------- merge exchange 1 ----------------
            gmax = smp.tile([128, 3], F32, tag="gmax")
            v.tensor_tensor(out=gmax[:], in0=R1[:, 0:3], in1=R1[:, 8:11], op=Alu.max)
            armax = smp.tile([128, 3], F32, tag="armax")
            gp.partition_all_reduce(
                armax[:], gmax[:], channels=128, reduce_op=bass_isa.ReduceOp.max
            )
            v.tensor_scalar(armax[:], armax[:], 1.0e-6, None, op0=Alu.max)
            ninv = smp.tile([128, 3], F32, tag="ninv")
            v.reciprocal(ninv[:], armax[:])
            v.tensor_scalar(ninv[:], ninv[:], -1.0, None, op0=Alu.mult)
            ninvd = ninv[:, 0:1]
            ninvs = ninv[:, 1:2]
            ninvt = ninv[:, 2:3]

            # merged dist top-4 -> dyn_k thresholds (km1)
            cg2 = smp.tile([128, 8], F32, tag="cg2")
            v.tensor_copy(cg2[:, 0:4], R1[:, 4:8])
            v.tensor_copy(cg2[:, 4:8], R1[:, 12:16])
            nd8g = smp.tile([128, 8], F32, tag="nd8g")
            v.max(out=nd8g[:], in_=cg2[:])
            dq = smp.tile([T, 4], F32, tag="dq")
            v.tensor_scalar(dq[:], nd8g[0:T, 0:4], nvlenp, None, op0=Alu.mult)
            numq = smp.tile([T, 4], F32, tag="numq")
            v.tensor_scalar(numq[:], dq[:], -1.0, thirty, op0=Alu.mult, op1=Alu.add)
            denq = smp.tile([T, 4], F32, tag="denq")
            v.tensor_scalar(denq[:], dq[:], thirty_eps, None, op0=Alu.add)
            v.reciprocal(denq[:], denq[:])
            v.tensor_tensor(out=numq[:], in0=numq[:], in1=denq[:], op=Alu.mult)
            v.tensor_scalar(numq[:], numq[:], 0.0, None, op0=Alu.max)
            s4 = smp.tile([T, 1], F32, tag="s4")
            v.tensor_reduce(s4[:], numq[:], axis=AX.X, op=Alu.add)
            km1 = smp.tile([T, 1], F32, tag="km1")
            g3t = smp.tile([T, 1], F32, tag="g3t")
            v.tensor_scalar(km1[:], s4[:], 2.0, None, op0=Alu.is_ge)
            v.tensor_scalar(g3t[:], s4[:], 3.0, None, op0=Alu.is_ge)
            v.tensor_tensor(out=km1[:], in0=km1[:], in1=g3t[:], op=Alu.add)
            v.tensor_scalar(g3t[:], s4[:], 4.0, None, op0=Alu.is_ge)
            v.tensor_tensor(out=km1[:], in0=km1[:], in1=g3t[:], op=Alu.add)

            # ---------------- phase 2: negated cost ----------------
            # per-4-tile-group cost chain pipelined with its fold transposes:
            # group g's PE transposes run while group g+1's cost computes.
            a_ = bigp.tile([128, 2 * NTT], F32)
            b_ = bigp.tile([128, 2 * NTT], F32)
            c_ = bigp.tile([128, 2 * NTT], F32)
            sq = b_
            negcost = bigp.tile([128, 2 * NTT], F32)
            fold_nc = bigp.tile([128, 1024], F32, tag="foldednc")
            for g in range(2 * NG):
                gs, ge = g * GT * T, (g + 1) * GT * T
                ag = a_[:, gs:ge]
                sc.activation(ag, dist_all[:, gs:ge], Act.Copy, bias=1.01, scale=ninvd)
                sc.activation(
                    b_[:, gs:ge], sd_all[:, gs:ge], Act.Copy, bias=1.01, scale=ninvs
                )
                v.tensor_scalar(
                    c_[:, gs:ge], td_all[:, gs:ge], ninvt, 1.01,
                    op0=Alu.mult, op1=Alu.add,
                )
                v.tensor_tensor(out=ag, in0=ag, in1=b_[:, gs:ge], op=Alu.mult)
                v.tensor_tensor(out=ag, in0=ag, in1=c_[:, gs:ge], op=Alu.mult)
                v.tensor_scalar(ag, ag, -1.0e14, 1.0e14, op0=Alu.max, op1=Alu.min)
                sc.activation(sq[:, gs:ge], ag, Act.Square)
                v.scalar_tensor_tensor(
                    negcost[:, gs:ge], sq[:, gs:ge], 3.0, cls_all[:, gs:ge],
                    op0=Alu.mult, op1=Alu.subtract,
                )
                psg = psF.tile([T, 512], F32, tag="psg", name=f"psgc{g}")
                for j in range(GT):
                    i = GT * g + j
                    te.transpose(
                        psg[:, j * 128 : (j + 1) * 128],
                        negcost[:, i * T : (i + 1) * T],
                        ident[:],
                    )
                dst = fold_nc[
                    (g % 4) * 32 : (g % 4) * 32 + T,
                    (g // 4) * 512 : (g // 4) * 512 + 512,
                ]
                if g % 2 == 0:
                    sc.activation(dst, psg[:], Act.Copy)
                else:
                    v.tensor_copy(dst, psg[:])

            nc8 = fold_tail(fold_nc)

            # ck-independent selection pieces (own half only)
            nv3 = negcost[:, 0:NTT].rearrange("p (i t) -> p i t", t=T)
            nmax = smp.tile([128, NT], F32, tag="nmax")
            v.tensor_reduce(nmax[:], nv3, axis=AX.X, op=Alu.max)
            oh = bigp.tile([128, NTT], F32)
            oh3 = oh[:].rearrange("p (i t) -> p i t", t=T)
            v.tensor_tensor(out=oh3, in0=nv3, in1=b3(nmax[:], "t"), op=Alu.is_equal)
            v.tensor_scalar(oh[:], oh[:], -1.0e9, 1.0e9, op0=Alu.mult, op1=Alu.add)
            v.tensor_tensor(out=oh3, in0=oh3, in1=b3(origt[:], "tile"), op=Alu.add)
            idx2 = smp.tile([128, NT], F32, tag="idx2")
            v.tensor_reduce(idx2[:], oh3, axis=AX.X, op=Alu.min)

            # ck threshold per target from the (already global) cost top-4
            nc8g = nc8
            eqk = smp.tile([T, 4], F32, tag="eqk")
            v.tensor_scalar(eqk[:], iotaq, km1[:], None, op0=Alu.is_equal)
            v.tensor_tensor(out=eqk[:], in0=eqk[:], in1=nc8g[:, 0:4], op=Alu.mult)
            ck = smp.tile([T, 1], F32, tag="ck")
            v.tensor_reduce(ck[:], eqk[:], axis=AX.X, op=Alu.add)
            v.tensor_tensor(out=ck[:], in0=ck[:], in1=maskcol, op=Alu.mult)
            v.tensor_tensor(out=ck[:], in0=ck[:], in1=mb, op=Alu.add)
            ckr = smp.tile([1, T], F32, tag="ckr")
            nc.sync.dma_start(out=ckr[:], in_=ck[:])
            ckb = smp.tile([128, T], F32, tag="ckb")
            pe_bcast(ckb[:], ckr[:], T, "ck")

            # ---------------- selection / conflict / outputs ----------
            mm = bigp.tile([128, NTT], F32)
            mm3 = mm[:].rearrange("p (i t) -> p i t", t=T)
            v.tensor_tensor(out=mm3, in0=nv3, in1=b3(ckb[:], "tile"), op=Alu.is_ge)
            rs = smp.tile([128, NT], F32, tag="rs")
            v.tensor_reduce(rs[:], mm3, axis=AX.X, op=Alu.add)
            # assigned output first: its transpose/cast/DMA can run while the
            # matched-index chain below still computes.
            asum = smp.tile([128, NT], F32, tag="asum")
            v.tensor_scalar(asum[:], rs[:], 1.0, None, op0=Alu.is_ge)
            p_aT = psF.tile([NT, 128], F32, tag="psg", name="p_aT")
            te.transpose(p_aT[:], asum[:], ident[:])
            aTf = smp.tile([NT, 128], F32, tag="aTf")
            sc.copy(aTf[:], p_aT[:])
            aT8 = smp.tile([NT, 128], U8, tag="aT8")
            v.tensor_copy(aT8[:], aTf[:])
            nc.sync.dma_start(out=bass.AP(asn_d, 0, [[128, NT], [1, 128]]), in_=aT8[:])
            conf = smp.tile([128, NT], F32, tag="conf")
            v.tensor_scalar(conf[:], rs[:], 1.0, None, op0=Alu.is_gt)
            # first selected target (mm) and argmin-cost target (oh)
            v.tensor_scalar(mm[:], mm[:], -1.0e9, 1.0e9, op0=Alu.mult, op1=Alu.add)
            v.tensor_tensor(out=mm3, in0=mm3, in1=b3(origt[:], "tile"), op=Alu.add)
            idx1 = smp.tile([128, NT], F32, tag="idx1")
            v.tensor_reduce(idx1[:], mm3, axis=AX.X, op=Alu.min)
            # idx = conf ? idx2 : idx1   (conflict rows keep argmin-cost)
            idx2b = smp.tile([128, NT], F32, tag="idx2b")
            v.tensor_tensor(out=idx2b[:], in0=idx2[:], in1=idx1[:], op=Alu.subtract)
            v.tensor_tensor(out=idx2b[:], in0=idx2b[:], in1=conf[:], op=Alu.mult)
            idxm = smp.tile([128, NT], F32, tag="idxm")
            v.tensor_tensor(out=idxm[:], in0=idx1[:], in1=idx2b[:], op=Alu.add)
            # matched = assigned * (idx+1) - 1 ; then transpose for output
            v.tensor_scalar(idxm[:], idxm[:], 1.0, None, op0=Alu.add)
            v.tensor_tensor(out=idxm[:], in0=idxm[:], in1=asum[:], op=Alu.mult)
            v.tensor_scalar(idxm[:], idxm[:], -1.0, None, op0=Alu.add)

            p_mT = psF.tile([NT, 128], F32, tag="psg", name="p_mT")
            te.transpose(p_mT[:], idxm[:], ident[:])
            mTf = smp.tile([NT, 128], F32, tag="mTf")
            sc.copy(mTf[:], p_mT[:])
            mT32 = smp.tile([NT, 128], I32, tag="mT32")
            v.tensor_copy(mT32[:], mTf[:])
            nc.sync.dma_start(out=bass.AP(mat_d, 0, [[128, NT], [1, 128]]), in_=mT32[:])

    nc.compile()
    return nc


_CACHE: dict[tuple, bass.Bass] = {}


def _get_nc(img_w: float, Tc: int) -> bass.Bass:
    key = (img_w, Tc)
    if key not in _CACHE:
        _CACHE[key] = build(img_w, Tc)
    return _CACHE[key]


def _compact(targets, masks):
    """Keep only valid target columns, padded to the batch max count."""
    B = targets.shape[0]
    counts = [int(masks[b].sum()) for b in range(B)]
    Tc = max(1, max(counts))
    ct = np.zeros((B, Tc, 78), np.float32)
    cm = np.zeros((B, Tc), np.int32)
    ot = np.zeros((B, Tc), np.float32)
    for b in range(B):
        idx = np.nonzero(masks[b])[0]
        k = len(idx)
        if k:
            ct[b, :k] = targets[b, idx]
            ot[b, :k] = idx.astype(np.float32)
            cm[b, :k] = 1
    return Tc, ct, cm, ot


def _in_maps(preds, targets, masks, img_w=800.0, n_cores=8):
    B = preds.shape[0]
    preds = np.asarray(preds, np.float32)
    Tc, ct, cm, ot = _compact(np.asarray(targets), np.asarray(masks))
    T = Tc
    ident = np.eye(128, dtype=np.float32)
    f32 = np.float32
    w = f32(img_w)
    maps = []
    perimg = {}
    for b in range(B):
        tx = ct[b][:, 6:78].astype(f32)  # [T, S]
        mv = ((tx >= f32(0.0)) & (tx < w)).astype(f32)
        txn = (tx * mv).astype(f32)
        inval = (f32(1.0) - mv).astype(f32)
        vcnt = mv.sum(axis=1, dtype=f32)  # [T]
        vlenp = (np.maximum(vcnt, f32(1.0)) + f32(1.0e-6)).astype(f32)
        invvlen = (f32(1.0) / vlenp).astype(f32)
        valid = cm[b].astype(f32)
        trows = np.concatenate(
            [
                ct[b][:, 1].astype(f32),  # lab
                ct[b][:, 2].astype(f32),  # sx
                ct[b][:, 3].astype(f32),  # sy
                ct[b][:, 4].astype(f32),  # th
                valid,
                invvlen,
                (valid * f32(2.0e30) - f32(1.0e30)).astype(f32),  # maskmin
                (valid * f32(-1.0e30) + f32(1.0e30)).astype(f32),  # bigmask
            ]
        ).reshape(1, 8 * T)
        tcols = np.zeros((T, 16), f32)
        tcols[:, 0:4] = np.arange(4, dtype=f32)[None, :]
        tcols[:, 4] = (vcnt * f32(30.0)).astype(f32)
        tcols[:, 5] = (vcnt * f32(30.0) + f32(1.0e-9)).astype(f32)
        tcols[:, 6] = (-vlenp).astype(f32)
        tcols[:, 7] = valid
        tcols[:, 8] = (valid * f32(-1.0e30) + f32(1.0e30)).astype(f32)
        perimg[b] = {
            "ident": ident,
            "txball": np.ascontiguousarray(
                np.tile(txn.reshape(1, T * S), (128, 1)), f32
            ),
            "trowsb": np.ascontiguousarray(np.tile(trows, (128, 1)), f32),
            "invms": np.ascontiguousarray(inval.T, f32),
            "tcols": np.ascontiguousarray(tcols, f32),
            "origt": np.ascontiguousarray(np.tile(ot[b][None, :], (128, 1)), f32),
        }
    def _half_cols(b, h):
        p = preds[b][h * NH : (h + 1) * NH]
        pxall = (
            p[:, 6:78].reshape(NT, 128, S).transpose(1, 0, 2).reshape(128, NT * S)
        )
        pcols = p[:, 0:5].reshape(NT, 128, 5).transpose(1, 2, 0).reshape(128, 5 * NT)
        return pxall, pcols

    for c in range(n_cores):
        b, h = c // 2, c % 2
        pxall, pcols = _half_cols(b, h)
        _, pcolsp = _half_cols(b, 1 - h)
        maps.append(
            {
                "pxall": np.ascontiguousarray(pxall, np.float32),
                "pcols": np.ascontiguousarray(pcols, np.float32),
                "pcolsp": np.ascontiguousarray(pcolsp, np.float32),
                **perimg[b],
            }
        )
    return Tc, maps


def _assemble(results, B):
    assigned = np.stack(
        [
            np.concatenate([results[2 * b]["assigned"], results[2 * b + 1]["assigned"]])
            for b in range(B)
        ]
    ).astype(bool)
    matched = np.stack(
        [
            np.concatenate([results[2 * b]["matched"], results[2 * b + 1]["matched"]])
            for b in range(B)
        ]
    ).astype(np.int32)
    return assigned, matched


def kernel(preds, targets, masks, img_w, img_h):
    del img_h
    B = preds.shape[0]
    Tc, maps = _in_maps(preds, targets, masks, img_w=float(img_w))
    nc = _get_nc(float(img_w), Tc)
    from concourse.bass_utils import run_bass_kernel_spmd

    res = run_bass_kernel_spmd(nc, maps, list(range(8)))
    return _assemble(res.results, B)
